# revision 48
# baseline (speedup 1.0000x reference)
# Multi-head causal self-attention (B=2, S=2048, D=1024, H=16, Dh=64) on 8
# Trainium2 NeuronCores.
#
# Sharding: core i -> (batch b = i // 4, head-group g = i % 4). Each core
# computes attention for its batch's 4 heads (feature columns 256g:256g+256 of
# the QKV projections, rows 256g:256g+256 of Wo) and produces a partial
# out-projection [S, D]. Host sums the 4 partials per batch and adds bo.
#
# All matmul operands are bf16 (fp32 PSUM accumulation); ~4e-3 rel error,
# well under the 2e-2 gate.
#
# Schedule: ScalarE's exp stream is the attention-phase bottleneck (~1.1us
# per k-block covering both heads of a pair, (N+352)/1.2 ns) and every engine
# queue is in-order, so a dependency-stalled op head-of-line-blocks its
# engine. The emission therefore:
#   * software-pipelines the chunks: chunk c+1's projections (pure PE work)
#     are emitted inside chunk c's attention so the PE chews projections
#     while ScalarE grinds exps -- without this the kernel alternates
#     PE-bound proj phases (ScalarE idle 8-12us each) and ScalarE-bound
#     attention phases;
#   * within each attention call emits all scores+exp first, then all ctx
#     (a ctx matmul stalled on the previous pair's normalize chain must not
#     head-of-line-block the scores that feed ScalarE);
#   * host pre-shapes every DMA to be fully contiguous, weights arrive in
#     consumption order, and a tiny exp at t=0 preloads the ACT table set;
#   * dummy matmuls at t=0 keep the PE busy until the first DMAs land so the
#     HAM clock-gate (4/8 -> 8/8 after ~3.4us of sustained activity) is warm
#     from the first real matmul.
#
# Per-core dataflow details:
#   QT = Wq_s^T xT + bq [256, S]: PSUM->SBUF move + bias ride one DVE
#   tensor_scalar_add (keeps ScalarE free for exp). KT [256, S]: K's bias is
#   dropped -- (q+bq).(k+bk) differs from (q+bq).k by a per-query-row
#   constant, which softmax cancels. Head pair p keeps head 2p on partitions
#   0:64, head 2p+1 on 64:128. V = xT^T Wv_s + bv [S, 256], augmented with a
#   ones column per head ([V_h | 1]) so the attention matmul also accumulates
#   the softmax denominator. Scores: two CONCURRENT K=64 row-tiled matmuls
#   (tile_position (0,0)/(64,0)) -> one 2-bank PSUM tile; ONE exp covers both
#   heads; scores pre-scaled by 1/sqrt(Dh) via host-side Wq scaling (small
#   enough that max-subtraction is unnecessary). Causality = skip k>q blocks
#   + one triangular mask multiply (both heads) on diagonal blocks.
#   Normalize: recip(denom) on DVE, partition-broadcast via two K=1
#   column-tiled matmuls, multiply -> ctxT bf16.

import numpy as np
import ml_dtypes

import concourse.bass as bass
import concourse.mybir as mybir
import concourse.tile as tile
from concourse import bacc
from concourse.bass_utils import run_bass_kernel_spmd
from concourse.masks import make_upper_triangular

F32 = mybir.dt.float32
BF16 = mybir.dt.bfloat16

B, S, D = 2, 2048, 1024
H, DH = 16, 64
NCORES = 8
GROUPS = 4               # head-groups (tensor parallel)
HG = H // GROUPS         # 4 heads per group
NPAIR = HG // 2          # 2 head-pairs per group
FEAT = HG * DH           # 256 features per group
SCALE = 1.0 / 8.0        # 1/sqrt(DH), folded into Wq/bq on host

CHUNK = 512              # seq chunk (PSUM bank = 512 fp32)
NSUB = CHUNK // 128      # 4 seq subtiles per chunk
NCHUNK = S // CHUNK      # 4
KD = D // 128            # 8 k-tiles over D
MT = FEAT // 128         # 2 feature M-tiles per group (m-tile == head-pair)


def _emit(tc):
    nc = tc.nc
    # host pre-shapes everything into SBUF layout -> contiguous DMAs
    xt_d = nc.dram_tensor("xt", [128, NCHUNK, KD, CHUNK], BF16,
                          kind="ExternalInput").ap()
    wq = nc.dram_tensor("wq", [128, KD, MT, 128], BF16, kind="ExternalInput").ap()
    wk = nc.dram_tensor("wk", [128, KD, MT, 128], BF16, kind="ExternalInput").ap()
    wv = nc.dram_tensor("wv", [128, KD, FEAT], BF16, kind="ExternalInput").ap()
    wo = nc.dram_tensor("wo", [128, MT, D], BF16, kind="ExternalInput").ap()
    bq = nc.dram_tensor("bq", [128, MT], F32, kind="ExternalInput").ap()
    bv = nc.dram_tensor("bv", [128, HG, DH], F32, kind="ExternalInput").ap()
    out = nc.dram_tensor("out", [S, D], BF16, kind="ExternalOutput").ap()

    consts = tc.alloc_tile_pool(name="consts", bufs=1)
    weights = tc.alloc_tile_pool(name="weights", bufs=1)
    persist = tc.alloc_tile_pool(name="persist", bufs=1)
    qt_pool = tc.alloc_tile_pool(name="qt", bufs=2)
    et_pool = tc.alloc_tile_pool(name="et", bufs=8)
    rc_pool = tc.alloc_tile_pool(name="rc", bufs=2)
    ob_pool = tc.alloc_tile_pool(name="ob", bufs=2)
    work_ps = tc.alloc_tile_pool(name="work_ps", bufs=2, space="PSUM")
    sp_ps = tc.alloc_tile_pool(name="sp_ps", bufs=2, space="PSUM")
    cx_ps = tc.alloc_tile_pool(name="cx_ps", bufs=1, space="PSUM")

    # ---- preload the ACT exp table set while DMAs are in flight
    dum = consts.tile([1, 2], F32)
    nc.gpsimd.memset(dum, 0.0)
    nc.scalar.activation(dum[0:1, 1:2], dum[0:1, 0:1],
                         mybir.ActivationFunctionType.Exp)

    # ---- x^T chunk 0 first so projections can start ASAP
    xtall = persist.tile([128, NCHUNK, KD, CHUNK], BF16)
    nc.sync.dma_start(xtall[:, 0], xt_d[:, 0])

    # ---- weights (in first-consumption order)
    wq_sb = weights.tile([128, KD, MT, 128], BF16)
    nc.sync.dma_start(wq_sb, wq)
    bqt = weights.tile([128, MT], F32)
    nc.sync.dma_start(bqt, bq)
    wk_sb = weights.tile([128, KD, MT, 128], BF16)
    nc.sync.dma_start(wk_sb, wk)
    wv_sb = weights.tile([128, KD, FEAT], BF16)
    nc.sync.dma_start(wv_sb, wv)
    bvb = weights.tile([128, HG, DH], F32)
    nc.sync.dma_start(bvb, bv)

    # rest of x^T (per-chunk DMAs so later chunks can't delay earlier ones);
    # wo interleaved -- first needed only at outproj(0)
    for c in range(1, NCHUNK):
        nc.sync.dma_start(xtall[:, c], xt_d[:, c])
    wo_sb = weights.tile([128, MT, D], BF16)
    nc.sync.dma_start(wo_sb, wo)

    # ---- constants
    onesf = consts.tile([128, 64], F32)
    nc.vector.memset(onesf, 1.0)
    # tri[k, q] = 1 if q >= k else 0; two copies side by side so ONE DVE op
    # masks both heads of a pair
    tri = consts.tile([128, 128], BF16)
    make_upper_triangular(nc, tri, val=1.0, diag=True)
    tri2 = consts.tile([128, 2, 128], BF16)
    nc.vector.tensor_copy(tri2[:, 0, :], tri)
    nc.vector.tensor_copy(tri2[:, 1, :], tri)
    wrm = consts.tile([128, CHUNK], BF16)
    nc.gpsimd.memset(wrm, 0.0)
    ones64 = consts.tile([1, 64], BF16)
    nc.vector.memset(ones64, 1.0)

    # ---- HAM warmup, sized to end just as the first weights/x slices arrive
    # (~13.7us; DMA-BW-bound) -- ending early lets the HAM MID window
    # re-throttle and the first chunk runs cold.
    for _ in range(26):
        wp = work_ps.tile([128, CHUNK], F32, tag="w", name="wp")
        nc.tensor.matmul(wp[:, 0:256], wrm[:, 0:128], wrm[:, 0:256],
                         start=True, stop=True)

    # ---- persistent activations
    # K^T pair-packed: pair p = heads (2p, 2p+1) on partitions 0:64 / 64:128
    kt2 = persist.tile([128, NPAIR, S], BF16)
    vaug = persist.tile([128, S // 128, HG, DH + 1], BF16)  # [V_h | 1] per head
    ctxT = persist.tile([128, MT, S], BF16)   # normalized ctx^T
    nc.vector.tensor_copy(vaug[:, :, :, DH],
                          onesf.rearrange("p (a b) -> p a b", a=S // 128))

    def proj_q(c, qt):
        for m in range(MT):
            ps = work_ps.tile([128, CHUNK], F32, tag="w", name="ps")
            for k in range(KD):
                nc.tensor.matmul(ps, wq_sb[:, k, m, :], xtall[:, c, k, :],
                                 start=(k == 0), stop=(k == KD - 1))
            nc.vector.tensor_scalar_add(qt[:, m, :], ps, bqt[:, m:m + 1])

    def proj_k(c):
        cs = c * CHUNK
        for m in range(MT):
            ps = work_ps.tile([128, CHUNK], F32, tag="w", name="ps")
            for k in range(KD):
                nc.tensor.matmul(ps, wk_sb[:, k, m, :], xtall[:, c, k, :],
                                 start=(k == 0), stop=(k == KD - 1))
            nc.vector.tensor_copy(kt2[:, m, cs:cs + CHUNK], ps)

    def proj_v(c, ts):
        for t in ts:
            gt = c * NSUB + t
            ps = work_ps.tile([128, CHUNK], F32, tag="w", name="ps")
            for k in range(KD):
                nc.tensor.matmul(ps[:, 0:FEAT],
                                 xtall[:, c, k, t * 128:(t + 1) * 128],
                                 wv_sb[:, k, :],
                                 start=(k == 0), stop=(k == KD - 1))
            nc.vector.tensor_add(
                vaug[:, gt, :, 0:DH],
                ps[:, 0:FEAT].rearrange("p (h f) -> p h f", h=HG), bvb)

    def attn(c, p, qt, cxa, cxb, j0, j1, first, last):
        # scores+exp phase first, then ctx phase: a ctx matmul stalled on the
        # previous pair's normalize (cx-bank WAR) must not head-of-line-block
        # the scores feeding ScalarE. et_pool provides the elasticity.
        cs = c * CHUNK
        ets = []
        for j in range(j0, j1):
            lv = max(0, 128 * j - cs)   # first valid q (chunk-local)
            nq = CHUNK - lv
            sp = sp_ps.tile([128, 2, CHUNK], F32, tag="sp", name="sp")
            nc.tensor.matmul(sp[:, 0, 0:nq],
                             kt2[0:64, p, 128 * j:128 * (j + 1)],
                             qt[0:64, p, lv:CHUNK], start=True, stop=True)
            nc.tensor.matmul(sp[:, 1, 0:nq],
                             kt2[64:128, p, 128 * j:128 * (j + 1)],
                             qt[64:128, p, lv:CHUNK], start=True, stop=True)
            et = et_pool.tile([128, 2, CHUNK], BF16, name="et")
            nc.scalar.activation(et[:, :, 0:nq], sp[:, :, 0:nq],
                                 mybir.ActivationFunctionType.Exp)
            if j >= c * NSUB:  # diagonal block: causal triangular mask
                nc.vector.tensor_mul(et[:, :, 0:128], et[:, :, 0:128], tri2)
            ets.append((j, lv, nq, et))
        for j, lv, nq, et in ets:
            nc.tensor.matmul(cxa[:, lv:CHUNK], vaug[:, j, 2 * p, :],
                             et[:, 0, 0:nq],
                             start=(first and j == j0),
                             stop=(last and j == j1 - 1),
                             skip_group_check=True)
            nc.tensor.matmul(cxb[:, lv:CHUNK], vaug[:, j, 2 * p + 1, :],
                             et[:, 1, 0:nq],
                             start=(first and j == j0),
                             stop=(last and j == j1 - 1),
                             skip_group_check=True)

    def normalize(c, p, cxa, cxb):
        """Normalize both heads of a pair. Denominator rows (bf16) -> two
        column-tiled K=1 broadcast matmuls into one PSUM bank -> one 128-lane
        reciprocal -> two multiplies (cx PSUM x bcs, as the hardware
        partition-shift supports PSUM-side reads)."""
        cs = c * CHUNK
        rda = rc_pool.tile([1, CHUNK], BF16, tag="rda")
        nc.vector.tensor_copy(rda, cxa[DH:DH + 1, :])
        rdb = rc_pool.tile([1, CHUNK], BF16, tag="rdb")
        nc.vector.tensor_copy(rdb, cxb[DH:DH + 1, :])
        bcd = work_ps.tile([128, CHUNK], F32, tag="w", name="bcd")
        nc.tensor.matmul(bcd[0:64, :], ones64, rda, start=True, stop=True)
        nc.tensor.matmul(bcd[64:128, :], ones64, rdb, start=True, stop=True,
                         skip_group_check=True)
        bcs = rc_pool.tile([128, CHUNK], F32, tag="bcs")
        nc.vector.reciprocal_approx_fast(bcs, bcd)
        nc.vector.tensor_mul(ctxT[0:64, p, cs:cs + CHUNK],
                             cxa[0:DH, :], bcs[0:64, :])
        nc.vector.tensor_mul(ctxT[64:128, p, cs:cs + CHUNK],
                             cxb[0:DH, :], bcs[64:128, :])

    def outproj(c, ts=range(NSUB), final=False):
        for t in ts:
            gt = c * NSUB + t
            ob = ob_pool.tile([128, D], BF16)
            for n in range(D // 512):
                op = work_ps.tile([128, CHUNK], F32, tag="w", name="op")
                for k in range(MT):
                    nc.tensor.matmul(
                        op,
                        ctxT[:, k, gt * 128:(gt + 1) * 128],
                        wo_sb[:, k, 512 * n:512 * (n + 1)],
                        start=(k == 0), stop=(k == MT - 1))
                # epilogue: alternate the PSUM->SBUF casts between DVE and the
                # (idle by then) ScalarE to halve the tail's cast chain
                if final and n == 1:
                    nc.scalar.copy(ob[:, 512 * n:512 * (n + 1)], op)
                else:
                    nc.vector.tensor_copy(ob[:, 512 * n:512 * (n + 1)], op)
            nc.sync.dma_start(out[gt * 128:(gt + 1) * 128, :], ob)

    # ---- main pipeline: chunk 0's projections in the prologue, then chunk
    # c's attention overlapping chunk c+1's projections + chunk c-1's outproj
    # The last chunk keeps its own K/V projections (emitted during its long
    # off-diagonal phases, before the diagonal needs them): by chunk 3 all
    # other fill work is exhausted and exp-paced attention leaves the PE
    # ~33% idle -- concentrated idle re-throttles the HAM clock-gate and the
    # whole tail runs at 1.2GHz. Keeping proj_k/v(3) local spreads the idle
    # across chunks 2+3 in sub-threshold dribbles instead.
    LASTC = NCHUNK - 1
    qt = qt_pool.tile([128, MT, CHUNK], BF16, name="qt")
    proj_q(0, qt)
    proj_k(0)
    proj_v(0, range(NSUB))
    for c in range(NCHUNK):
        jd0, jd1 = c * NSUB, (c + 1) * NSUB
        cxa0 = cx_ps.tile([DH + 1, CHUNK], F32, tag="cxa", name="cxa0")
        cxb0 = cx_ps.tile([DH + 1, CHUNK], F32, tag="cxb", name="cxb0")
        attn(c, 0, qt, cxa0, cxb0, 0, jd0, True, False)      # off-diagonal
        if c == LASTC:
            proj_k(c)
            proj_v(c, range(NSUB))
        attn(c, 0, qt, cxa0, cxb0, jd0, jd1, c == 0, True)   # diagonal
        normalize(c, 0, cxa0, cxb0)
        if c + 1 < NCHUNK:
            qt_next = qt_pool.tile([128, MT, CHUNK], BF16, name="qt")
            proj_q(c + 1, qt_next)
        else:
            qt_next = None
        cxa1 = cx_ps.tile([DH + 1, CHUNK], F32, tag="cxa", name="cxa1")
        cxb1 = cx_ps.tile([DH + 1, CHUNK], F32, tag="cxb", name="cxb1")
        attn(c, 1, qt, cxa1, cxb1, 0, jd0, True, False)
        if c + 1 < NCHUNK - 1:
            proj_k(c + 1)
        # deferred out-projections ride the chunks whose exp-paced attention
        # leaves the PE idle (chunk 1 is already PE-oversubscribed; chunks
        # 2-3 starve and their idle re-throttles the HAM clock-gate)
        if c == 2:
            outproj(0)
            outproj(1)
        elif c == 3:
            outproj(2)
        attn(c, 1, qt, cxa1, cxb1, jd0, jd1, c == 0, True)
        normalize(c, 1, cxa1, cxb1)
        if c + 1 < NCHUNK - 1:
            proj_v(c + 1, range(NSUB))
        qt = qt_next

    outproj(NCHUNK - 1, final=True)

    for p in (cx_ps, sp_ps, work_ps, ob_pool, rc_pool, et_pool, qt_pool,
              persist, weights, consts):
        p.release()


_BUILT = None


def _build():
    global _BUILT
    if _BUILT is None:
        nc = bacc.Bacc("TRN2", target_bir_lowering=False, debug=False,
                       num_devices=NCORES)
        with tile.TileContext(nc) as tc:
            _emit(tc)
        nc.compile()
        _BUILT = nc
    return _BUILT


def _bf16(a):
    return np.ascontiguousarray(a).astype(ml_dtypes.bfloat16)


def _shards(inputs):
    x = np.asarray(inputs["x"], dtype=np.float32)
    # [p, c, k, s] chunk-major transposed x per batch
    xts = [np.ascontiguousarray(
        x[b].T.reshape(KD, 128, NCHUNK, CHUNK).transpose(1, 2, 0, 3)
    ).astype(ml_dtypes.bfloat16) for b in range(B)]
    Wq = np.asarray(inputs["Wq"], np.float32)
    Wk = np.asarray(inputs["Wk"], np.float32)
    Wv = np.asarray(inputs["Wv"], np.float32)
    Wo = np.asarray(inputs["Wo"], np.float32)
    bq_ = np.asarray(inputs["bq"], np.float32)
    bv_ = np.asarray(inputs["bv"], np.float32)
    maps = []
    for core in range(NCORES):
        b, g = core // GROUPS, core % GROUPS
        f0 = g * FEAT
        m = {
            "xt": xts[b],
            # [p, k, m, f]
            "wq": _bf16((Wq[:, f0:f0 + FEAT] * SCALE)
                        .reshape(KD, 128, MT, 128).transpose(1, 0, 2, 3)),
            "wk": _bf16(Wk[:, f0:f0 + FEAT]
                        .reshape(KD, 128, MT, 128).transpose(1, 0, 2, 3)),
            # [p, k, f]
            "wv": _bf16(Wv[:, f0:f0 + FEAT]
                        .reshape(KD, 128, FEAT).transpose(1, 0, 2)),
            # [p, m, d]
            "wo": _bf16(Wo[f0:f0 + FEAT, :]
                        .reshape(MT, 128, D).transpose(1, 0, 2)),
            # [p, m]
            "bq": np.ascontiguousarray(
                (bq_[f0:f0 + FEAT] * SCALE).reshape(MT, 128).T),
            # [p, h, f] broadcast
            "bv": np.ascontiguousarray(np.broadcast_to(
                bv_[f0:f0 + FEAT].reshape(HG, DH), (128, HG, DH))),
        }
        maps.append(m)
    return maps


def kernel(trace=False, **inputs):
    nc = _build()
    res = run_bass_kernel_spmd(nc, _shards(inputs), core_ids=list(range(NCORES)),
                               trace=trace)
    partial = np.stack([np.asarray(r_["out"], np.float64)
                        for r_ in res.results])  # [8, S, D]
    acc = partial.reshape(B, GROUPS, S, D).sum(axis=1)
    acc += np.asarray(inputs["bo"], dtype=np.float64)
    out = acc.astype(np.float32)
    if trace:
        return out, res
    return out


# revision 49
# speedup vs baseline: 1.0071x; 1.0071x over previous
# Multi-head causal self-attention (B=2, S=2048, D=1024, H=16, Dh=64) on 8
# Trainium2 NeuronCores.
#
# Sharding: core i -> (batch b = i // 4, head-group g = i % 4). Each core
# computes attention for its batch's 4 heads (feature columns 256g:256g+256 of
# the QKV projections, rows 256g:256g+256 of Wo) and produces a partial
# out-projection [S, D]. Host sums the 4 partials per batch and adds bo.
#
# All matmul operands are bf16 (fp32 PSUM accumulation); ~4e-3 rel error,
# well under the 2e-2 gate.
#
# Schedule: ScalarE's exp stream is the attention-phase bottleneck (~1.1us
# per k-block covering both heads of a pair, (N+352)/1.2 ns) and every engine
# queue is in-order, so a dependency-stalled op head-of-line-blocks its
# engine. The emission therefore:
#   * software-pipelines the chunks: chunk c+1's projections (pure PE work)
#     are emitted inside chunk c's attention so the PE chews projections
#     while ScalarE grinds exps -- without this the kernel alternates
#     PE-bound proj phases (ScalarE idle 8-12us each) and ScalarE-bound
#     attention phases;
#   * within each attention call emits all scores+exp first, then all ctx
#     (a ctx matmul stalled on the previous pair's normalize chain must not
#     head-of-line-block the scores that feed ScalarE);
#   * host pre-shapes every DMA to be fully contiguous, weights arrive in
#     consumption order, and a tiny exp at t=0 preloads the ACT table set;
#   * dummy matmuls at t=0 keep the PE busy until the first DMAs land so the
#     HAM clock-gate (4/8 -> 8/8 after ~3.4us of sustained activity) is warm
#     from the first real matmul.
#
# Per-core dataflow details:
#   QT = Wq_s^T xT + bq [256, S]: PSUM->SBUF move + bias ride one DVE
#   tensor_scalar_add (keeps ScalarE free for exp). KT [256, S]: K's bias is
#   dropped -- (q+bq).(k+bk) differs from (q+bq).k by a per-query-row
#   constant, which softmax cancels. Head pair p keeps head 2p on partitions
#   0:64, head 2p+1 on 64:128. V = xT^T Wv_s + bv [S, 256], augmented with a
#   ones column per head ([V_h | 1]) so the attention matmul also accumulates
#   the softmax denominator. Scores: two CONCURRENT K=64 row-tiled matmuls
#   (tile_position (0,0)/(64,0)) -> one 2-bank PSUM tile; ONE exp covers both
#   heads; scores pre-scaled by 1/sqrt(Dh) via host-side Wq scaling (small
#   enough that max-subtraction is unnecessary). Causality = skip k>q blocks
#   + one triangular mask multiply (both heads) on diagonal blocks.
#   Normalize: recip(denom) on DVE, partition-broadcast via two K=1
#   column-tiled matmuls, multiply -> ctxT bf16.

import numpy as np
import ml_dtypes

import concourse.bass as bass
import concourse.mybir as mybir
import concourse.tile as tile
from concourse import bacc
from concourse.bass_utils import run_bass_kernel_spmd
from concourse.masks import make_upper_triangular

F32 = mybir.dt.float32
BF16 = mybir.dt.bfloat16

B, S, D = 2, 2048, 1024
H, DH = 16, 64
NCORES = 8
GROUPS = 4               # head-groups (tensor parallel)
HG = H // GROUPS         # 4 heads per group
NPAIR = HG // 2          # 2 head-pairs per group
FEAT = HG * DH           # 256 features per group
SCALE = 1.0 / 8.0        # 1/sqrt(DH), folded into Wq/bq on host

CHUNK = 512              # seq chunk (PSUM bank = 512 fp32)
NSUB = CHUNK // 128      # 4 seq subtiles per chunk
NCHUNK = S // CHUNK      # 4
KD = D // 128            # 8 k-tiles over D
MT = FEAT // 128         # 2 feature M-tiles per group (m-tile == head-pair)


def _emit(tc):
    nc = tc.nc
    # host pre-shapes everything into SBUF layout -> contiguous DMAs
    xt_d = nc.dram_tensor("xt", [128, NCHUNK, KD, CHUNK], BF16,
                          kind="ExternalInput").ap()
    wq = nc.dram_tensor("wq", [128, KD, MT, 128], BF16, kind="ExternalInput").ap()
    wk = nc.dram_tensor("wk", [128, KD, MT, 128], BF16, kind="ExternalInput").ap()
    wv = nc.dram_tensor("wv", [128, KD, FEAT], BF16, kind="ExternalInput").ap()
    wo = nc.dram_tensor("wo", [128, MT, D], BF16, kind="ExternalInput").ap()
    bq = nc.dram_tensor("bq", [128, MT], F32, kind="ExternalInput").ap()
    bv = nc.dram_tensor("bv", [128, HG, DH], F32, kind="ExternalInput").ap()
    out = nc.dram_tensor("out", [S, D], BF16, kind="ExternalOutput").ap()

    consts = tc.alloc_tile_pool(name="consts", bufs=1)
    weights = tc.alloc_tile_pool(name="weights", bufs=1)
    persist = tc.alloc_tile_pool(name="persist", bufs=1)
    qt_pool = tc.alloc_tile_pool(name="qt", bufs=2)
    et_pool = tc.alloc_tile_pool(name="et", bufs=8)
    rc_pool = tc.alloc_tile_pool(name="rc", bufs=2)
    ob_pool = tc.alloc_tile_pool(name="ob", bufs=2)
    work_ps = tc.alloc_tile_pool(name="work_ps", bufs=2, space="PSUM")
    sp_ps = tc.alloc_tile_pool(name="sp_ps", bufs=2, space="PSUM")
    cx_ps = tc.alloc_tile_pool(name="cx_ps", bufs=1, space="PSUM")

    # ---- preload the ACT exp table set while DMAs are in flight
    dum = consts.tile([1, 2], F32)
    nc.gpsimd.memset(dum, 0.0)
    nc.scalar.activation(dum[0:1, 1:2], dum[0:1, 0:1],
                         mybir.ActivationFunctionType.Exp)

    # ---- x^T chunk 0 first so projections can start ASAP
    xtall = persist.tile([128, NCHUNK, KD, CHUNK], BF16)
    nc.sync.dma_start(xtall[:, 0], xt_d[:, 0])

    # ---- weights (in first-consumption order)
    wq_sb = weights.tile([128, KD, MT, 128], BF16)
    nc.sync.dma_start(wq_sb, wq)
    bqt = weights.tile([128, MT], F32)
    nc.sync.dma_start(bqt, bq)
    wk_sb = weights.tile([128, KD, MT, 128], BF16)
    nc.sync.dma_start(wk_sb, wk)
    wv_sb = weights.tile([128, KD, FEAT], BF16)
    nc.sync.dma_start(wv_sb, wv)
    bvb = weights.tile([128, HG, DH], F32)
    nc.sync.dma_start(bvb, bv)

    # rest of x^T (per-chunk DMAs so later chunks can't delay earlier ones);
    # wo interleaved -- first needed only at outproj(0)
    for c in range(1, NCHUNK):
        nc.sync.dma_start(xtall[:, c], xt_d[:, c])
    wo_sb = weights.tile([128, MT, D], BF16)
    nc.sync.dma_start(wo_sb, wo)

    # ---- constants
    onesf = consts.tile([128, 64], F32)
    nc.vector.memset(onesf, 1.0)
    # tri[k, q] = 1 if q >= k else 0; two copies side by side so ONE DVE op
    # masks both heads of a pair
    tri = consts.tile([128, 128], BF16)
    make_upper_triangular(nc, tri, val=1.0, diag=True)
    tri2 = consts.tile([128, 2, 128], BF16)
    nc.vector.tensor_copy(tri2[:, 0, :], tri)
    nc.vector.tensor_copy(tri2[:, 1, :], tri)
    wrm = consts.tile([128, CHUNK], BF16)
    nc.gpsimd.memset(wrm, 0.0)
    ones64 = consts.tile([1, 64], BF16)
    nc.vector.memset(ones64, 1.0)

    # ---- HAM warmup, sized to end just as the first weights/x slices arrive
    # (~13.7us; DMA-BW-bound) -- ending early lets the HAM MID window
    # re-throttle and the first chunk runs cold.
    for _ in range(26):
        wp = work_ps.tile([128, CHUNK], F32, tag="w", name="wp")
        nc.tensor.matmul(wp[:, 0:256], wrm[:, 0:128], wrm[:, 0:256],
                         start=True, stop=True)

    # ---- persistent activations
    # K^T pair-packed: pair p = heads (2p, 2p+1) on partitions 0:64 / 64:128
    kt2 = persist.tile([128, NPAIR, S], BF16)
    vaug = persist.tile([128, S // 128, HG, DH + 1], BF16)  # [V_h | 1] per head
    ctxT = persist.tile([128, MT, S], BF16)   # normalized ctx^T
    nc.vector.tensor_copy(vaug[:, :, :, DH],
                          onesf.rearrange("p (a b) -> p a b", a=S // 128))

    def proj_q(c, qt):
        for m in range(MT):
            ps = work_ps.tile([128, CHUNK], F32, tag="w", name="ps")
            for k in range(KD):
                nc.tensor.matmul(ps, wq_sb[:, k, m, :], xtall[:, c, k, :],
                                 start=(k == 0), stop=(k == KD - 1))
            nc.vector.tensor_scalar_add(qt[:, m, :], ps, bqt[:, m:m + 1])

    def proj_k(c):
        cs = c * CHUNK
        for m in range(MT):
            ps = work_ps.tile([128, CHUNK], F32, tag="w", name="ps")
            for k in range(KD):
                nc.tensor.matmul(ps, wk_sb[:, k, m, :], xtall[:, c, k, :],
                                 start=(k == 0), stop=(k == KD - 1))
            nc.vector.tensor_copy(kt2[:, m, cs:cs + CHUNK], ps)

    def proj_v(c, ts):
        for t in ts:
            gt = c * NSUB + t
            ps = work_ps.tile([128, CHUNK], F32, tag="w", name="ps")
            for k in range(KD):
                nc.tensor.matmul(ps[:, 0:FEAT],
                                 xtall[:, c, k, t * 128:(t + 1) * 128],
                                 wv_sb[:, k, :],
                                 start=(k == 0), stop=(k == KD - 1))
            nc.vector.tensor_add(
                vaug[:, gt, :, 0:DH],
                ps[:, 0:FEAT].rearrange("p (h f) -> p h f", h=HG), bvb)

    def attn(c, p, qt, cxa, cxb, j0, j1, first, last):
        # scores+exp phase first, then ctx phase: a ctx matmul stalled on the
        # previous pair's normalize (cx-bank WAR) must not head-of-line-block
        # the scores feeding ScalarE. et_pool provides the elasticity.
        cs = c * CHUNK
        ets = []
        for j in range(j0, j1):
            lv = max(0, 128 * j - cs)   # first valid q (chunk-local)
            nq = CHUNK - lv
            sp = sp_ps.tile([128, 2, CHUNK], F32, tag="sp", name="sp")
            nc.tensor.matmul(sp[:, 0, 0:nq],
                             kt2[0:64, p, 128 * j:128 * (j + 1)],
                             qt[0:64, p, lv:CHUNK], start=True, stop=True)
            nc.tensor.matmul(sp[:, 1, 0:nq],
                             kt2[64:128, p, 128 * j:128 * (j + 1)],
                             qt[64:128, p, lv:CHUNK], start=True, stop=True)
            et = et_pool.tile([128, 2, CHUNK], BF16, name="et")
            nc.scalar.activation(et[:, :, 0:nq], sp[:, :, 0:nq],
                                 mybir.ActivationFunctionType.Exp)
            if j >= c * NSUB:  # diagonal block: causal triangular mask
                nc.vector.tensor_mul(et[:, :, 0:128], et[:, :, 0:128], tri2)
            ets.append((j, lv, nq, et))
        for j, lv, nq, et in ets:
            nc.tensor.matmul(cxa[:, lv:CHUNK], vaug[:, j, 2 * p, :],
                             et[:, 0, 0:nq],
                             start=(first and j == j0),
                             stop=(last and j == j1 - 1),
                             skip_group_check=True)
            nc.tensor.matmul(cxb[:, lv:CHUNK], vaug[:, j, 2 * p + 1, :],
                             et[:, 1, 0:nq],
                             start=(first and j == j0),
                             stop=(last and j == j1 - 1),
                             skip_group_check=True)

    def normalize(c, p, cxa, cxb):
        """Normalize both heads of a pair. Denominator rows (bf16) -> two
        column-tiled K=1 broadcast matmuls into one PSUM bank -> one 128-lane
        reciprocal -> two multiplies (cx PSUM x bcs, as the hardware
        partition-shift supports PSUM-side reads)."""
        cs = c * CHUNK
        rda = rc_pool.tile([1, CHUNK], BF16, tag="rda")
        nc.vector.tensor_copy(rda, cxa[DH:DH + 1, :])
        rdb = rc_pool.tile([1, CHUNK], BF16, tag="rdb")
        nc.vector.tensor_copy(rdb, cxb[DH:DH + 1, :])
        bcd = work_ps.tile([128, CHUNK], F32, tag="w", name="bcd")
        nc.tensor.matmul(bcd[0:64, :], ones64, rda, start=True, stop=True)
        nc.tensor.matmul(bcd[64:128, :], ones64, rdb, start=True, stop=True,
                         skip_group_check=True)
        bcs = rc_pool.tile([128, CHUNK], F32, tag="bcs")
        nc.vector.reciprocal_approx_fast(bcs, bcd)
        nc.vector.tensor_mul(ctxT[0:64, p, cs:cs + CHUNK],
                             cxa[0:DH, :], bcs[0:64, :])
        nc.vector.tensor_mul(ctxT[64:128, p, cs:cs + CHUNK],
                             cxb[0:DH, :], bcs[64:128, :])

    def outproj(c, ts=range(NSUB), final=False):
        for t in ts:
            gt = c * NSUB + t
            ob = ob_pool.tile([128, D], BF16)
            for n in range(D // 512):
                op = work_ps.tile([128, CHUNK], F32, tag="w", name="op")
                for k in range(MT):
                    nc.tensor.matmul(
                        op,
                        ctxT[:, k, gt * 128:(gt + 1) * 128],
                        wo_sb[:, k, 512 * n:512 * (n + 1)],
                        start=(k == 0), stop=(k == MT - 1))
                # epilogue: alternate the PSUM->SBUF casts between DVE and the
                # (idle by then) ScalarE to halve the tail's cast chain
                if final and n == 1:
                    nc.scalar.copy(ob[:, 512 * n:512 * (n + 1)], op)
                else:
                    nc.vector.tensor_copy(ob[:, 512 * n:512 * (n + 1)], op)
            nc.sync.dma_start(out[gt * 128:(gt + 1) * 128, :], ob)

    # ---- main pipeline: chunk 0's projections in the prologue, then chunk
    # c's attention overlapping chunk c+1's projections + chunk c-1's outproj
    # The last chunk keeps its own K/V projections (emitted during its long
    # off-diagonal phases, before the diagonal needs them): by chunk 3 all
    # other fill work is exhausted and exp-paced attention leaves the PE
    # ~33% idle -- concentrated idle re-throttles the HAM clock-gate and the
    # whole tail runs at 1.2GHz. Keeping proj_k/v(3) local spreads the idle
    # across chunks 2+3 in sub-threshold dribbles instead.
    LASTC = NCHUNK - 1
    qt = qt_pool.tile([128, MT, CHUNK], BF16, name="qt")
    proj_q(0, qt)
    proj_k(0)
    proj_v(0, range(NSUB))
    for c in range(NCHUNK):
        jd0, jd1 = c * NSUB, (c + 1) * NSUB
        cxa0 = cx_ps.tile([DH + 1, CHUNK], F32, tag="cxa", name="cxa0")
        cxb0 = cx_ps.tile([DH + 1, CHUNK], F32, tag="cxb", name="cxb0")
        attn(c, 0, qt, cxa0, cxb0, 0, jd0, True, False)      # off-diagonal
        if c == LASTC:
            proj_k(c)
            proj_v(c, range(NSUB))
        attn(c, 0, qt, cxa0, cxb0, jd0, jd1, c == 0, True)   # diagonal
        normalize(c, 0, cxa0, cxb0)
        if c + 1 < NCHUNK:
            qt_next = qt_pool.tile([128, MT, CHUNK], BF16, name="qt")
            proj_q(c + 1, qt_next)
        else:
            qt_next = None
        cxa1 = cx_ps.tile([DH + 1, CHUNK], F32, tag="cxa", name="cxa1")
        cxb1 = cx_ps.tile([DH + 1, CHUNK], F32, tag="cxb", name="cxb1")
        attn(c, 1, qt, cxa1, cxb1, 0, jd0, True, False)
        if c + 1 < NCHUNK - 1:
            proj_k(c + 1)
        if c > 0:
            outproj(c - 1)
        attn(c, 1, qt, cxa1, cxb1, jd0, jd1, c == 0, True)
        normalize(c, 1, cxa1, cxb1)
        if c + 1 < NCHUNK - 1:
            proj_v(c + 1, range(NSUB))
        qt = qt_next

    outproj(NCHUNK - 1, final=True)

    for p in (cx_ps, sp_ps, work_ps, ob_pool, rc_pool, et_pool, qt_pool,
              persist, weights, consts):
        p.release()


_BUILT = None


def _build():
    global _BUILT
    if _BUILT is None:
        nc = bacc.Bacc("TRN2", target_bir_lowering=False, debug=False,
                       num_devices=NCORES)
        with tile.TileContext(nc) as tc:
            _emit(tc)
        nc.compile()
        _BUILT = nc
    return _BUILT


def _bf16(a):
    return np.ascontiguousarray(a).astype(ml_dtypes.bfloat16)


def _shards(inputs):
    x = np.asarray(inputs["x"], dtype=np.float32)
    # [p, c, k, s] chunk-major transposed x per batch
    xts = [np.ascontiguousarray(
        x[b].T.reshape(KD, 128, NCHUNK, CHUNK).transpose(1, 2, 0, 3)
    ).astype(ml_dtypes.bfloat16) for b in range(B)]
    Wq = np.asarray(inputs["Wq"], np.float32)
    Wk = np.asarray(inputs["Wk"], np.float32)
    Wv = np.asarray(inputs["Wv"], np.float32)
    Wo = np.asarray(inputs["Wo"], np.float32)
    bq_ = np.asarray(inputs["bq"], np.float32)
    bv_ = np.asarray(inputs["bv"], np.float32)
    maps = []
    for core in range(NCORES):
        b, g = core // GROUPS, core % GROUPS
        f0 = g * FEAT
        m = {
            "xt": xts[b],
            # [p, k, m, f]
            "wq": _bf16((Wq[:, f0:f0 + FEAT] * SCALE)
                        .reshape(KD, 128, MT, 128).transpose(1, 0, 2, 3)),
            "wk": _bf16(Wk[:, f0:f0 + FEAT]
                        .reshape(KD, 128, MT, 128).transpose(1, 0, 2, 3)),
            # [p, k, f]
            "wv": _bf16(Wv[:, f0:f0 + FEAT]
                        .reshape(KD, 128, FEAT).transpose(1, 0, 2)),
            # [p, m, d]
            "wo": _bf16(Wo[f0:f0 + FEAT, :]
                        .reshape(MT, 128, D).transpose(1, 0, 2)),
            # [p, m]
            "bq": np.ascontiguousarray(
                (bq_[f0:f0 + FEAT] * SCALE).reshape(MT, 128).T),
            # [p, h, f] broadcast
            "bv": np.ascontiguousarray(np.broadcast_to(
                bv_[f0:f0 + FEAT].reshape(HG, DH), (128, HG, DH))),
        }
        maps.append(m)
    return maps


def kernel(trace=False, **inputs):
    nc = _build()
    res = run_bass_kernel_spmd(nc, _shards(inputs), core_ids=list(range(NCORES)),
                               trace=trace)
    partial = np.stack([np.asarray(r_["out"], np.float64)
                        for r_ in res.results])  # [8, S, D]
    acc = partial.reshape(B, GROUPS, S, D).sum(axis=1)
    acc += np.asarray(inputs["bo"], dtype=np.float64)
    out = acc.astype(np.float32)
    if trace:
        return out, res
    return out
